# revision 56
# baseline (speedup 1.0000x reference)
"""Trainium2 Bass kernel for nn_CapsuleLinear (k-means 'dot' routing, 3 iters).

Math (per example b):
  priors[o,i,v] = sum_l W[o,i,v,l] * x[b,i,l]
  out0 = mean_i priors
  3x: n = normalize(out); logits[o,i] = sum_v priors*n; probs = softmax_o(logits);
      out[o,v] = sum_i probs*priors
  result = squash(out) + bias

Sharding: data-parallel over batch B=64 across 8 cores (8 examples/core).

Per-core layout (P = 128 partitions = (i_p in 0..15, b in 0..7), p = i_p*8+b):
  priors SBUF fp16 [128, ib=32, v=16, o=64], full i = ib*16 + i_p.

v2 engine plan (vs the 216us baseline):
  - DMA on 3 queues (sync/scalar HWDGE + gpsimd SWDGE) with per-chunk weight
    tiles and split xdg tiles so the first priors matmul starts ~4us in
    instead of fencing on all DMAs (~15.7us).
  - Phase-1 PSUM->SBUF casts split ACT:DVE:POOL = 16:6:10 (DVE also owns the
    odd-ib out0 tree, ACT is fastest per cast, Pool is otherwise idle).
  - Routing iterations: the two big elementwise muls (priors*n, priors*probs)
    are split DVE 26 ibs / Pool(gpsimd) 6 ibs per mul -- DVE fp16 2x mode
    runs ~0.54ns/elem/lane, Pool ~2.0, so this shaves ~20% off the DVE-serial
    mul time which is the iteration critical path.
  - Softmax smalls (zs reduce, recip, rz fp16 cast, probs mul) stay on DVE,
    interleaved between mul slices in issue order; exp per 8-ib chunk on ACT
    reading logits straight from PSUM.
  - ACT activation tables: sqrt only appears in the per-iter norm chain; the
    loads (sqrt<->exp) land in ACT idle gaps rather than on the chain.
  - i-reduce (ones-matmul) issue order matches expected prod2 arrival
    (DVE ibs 0..19, 26..31, then Pool ibs 20..25) so the PE never stalls on
    an early-issued-but-late ib.
"""

import os

import numpy as np

import concourse.bacc as bacc
import concourse.bass_utils as _bu
import concourse.tile as tile
from concourse import mybir
from concourse.bass_utils import run_bass_kernel_spmd

# (walrus --enable-ldw-opt is incompatible with bass-emitted InstLdweights;
# the redundant per-matmul LDWEIGHTS stay.)

B, I, O, V, L = 64, 512, 64, 16, 8
NCORES = 8
BL = B // NCORES  # 8 examples per core
IB = I // 16  # 32 blocks of 16 i's
NQ = 4  # ib-chunks per pass
QIB = IB // NQ  # 8 ibs per chunk

f32 = mybir.dt.float32
f16 = mybir.dt.float16

# mul2 DVE slice bounds: first slice 4-ib so it starts as soon as the first
# half of chunk-0 probs lands; tail split small so the PE i-reduce drains
# right behind the last mul.
DVE_M2_SLICES = ((0, 4), (4, 8), (8, 16), (16, 24), (24, 28), (28, 30), (30, 32))
# phase-1 cast engine per ib ('a'=ACT, 'd'=DVE): gpsimd cannot read PSUM,
# so casts split ACT 20 / DVE 12; the odd-ib window adds go to gpsimd instead.
CAST_PAT = list("aaadadad") * 4

LAST_RESULT = None  # stash of BassKernelResults for test harness


def _build_kernel():
    nc = bacc.Bacc(
        "TRN2",
        target_bir_lowering=False,
        debug=False,
        enable_asserts=False,
        num_devices=NCORES,
    )
    w2_d = nc.dram_tensor("w2", [128, IB, O * V], f16, kind="ExternalInput")
    xdg_d = nc.dram_tensor("xdg", [128, IB, 128], f16, kind="ExternalInput")
    ones_d = nc.dram_tensor("onesd", [128, 128], f16, kind="ExternalInput")
    iden_d = nc.dram_tensor("idend", [128, 128], f16, kind="ExternalInput")
    bias_d = nc.dram_tensor("biasT", [V, O], f32, kind="ExternalInput")
    out_d = nc.dram_tensor("out", [BL, V, O], f32, kind="ExternalOutput")

    with tile.TileContext(nc) as tc:
        _body(nc, tc, w2_d, xdg_d, ones_d, iden_d, bias_d, out_d)
    nc.compile()
    return nc


def _body(nc, tc, w2_d, xdg_d, ones_d, iden_d, bias_d, out_d):
    AL = mybir.AluOpType
    X = mybir.AxisListType.X
    AF = mybir.ActivationFunctionType

    from contextlib import ExitStack

    with ExitStack() as ctx:
        big = ctx.enter_context(tc.tile_pool(name="big", bufs=1))
        wp = ctx.enter_context(tc.tile_pool(name="wp", bufs=6))
        sm = ctx.enter_context(tc.tile_pool(name="sm", bufs=1))
        # PSUM: ps_a holds phase-1 pp tiles (2 banks each, 3 deep) and the
        # per-iter logits halves; ps_o holds the out state (2 banks).
        ps_a = ctx.enter_context(tc.tile_pool(name="psa", bufs=3, space="PSUM"))
        ps_o = ctx.enter_context(tc.tile_pool(name="pso", bufs=1, space="PSUM"))

        # ---- persistent tiles ----
        priors = big.tile([128, IB, V, O], f16)
        prod = big.tile([128, IB, V, O], f16)
        probs = big.tile([128, IB, O], f16)
        elog = big.tile([128, IB, O], f16)
        zs = big.tile([128, IB], f16)
        rz = big.tile([128, IB], f32)
        ones_t = big.tile([128, 128], f16)
        iden_t = big.tile([128, 128], f16)
        bias_t = big.tile([BL, V, O], f32)
        xdgA = big.tile([128, 16, 128], f16)
        xdgB = big.tile([128, 16, 128], f16)
        # gpsimd tensor ops measured ~3.7us per 1024-elem/lane op (vs the
        # 2.0 modeled) and degrade concurrent DVE ops 15-30%, so all muls
        # stay on DVE; gpsimd only issues DMA and memset.
        ntile = big.tile([128, V, O], f16)

        bias8 = sm.tile([128, 1], f32, tag="b8")

        # ---- DMA: 3 queues, early-needed transfers first per queue ----
        # gpsimd(SWDGE): w c0 | ones iden bias | w c2
        # sync  (HWDGE): xdgA | w c1, c4, c6
        # scalar(HWDGE): xdgB | w c3, c5, c7
        # 16 chunks of 2 ibs each, round-robin over the three queues so the
        # stream arrival tracks the ib-order consumption (~100-145 GB/s per
        # queue, queue startup ~9-12us).
        w4 = []
        for c in range(16):
            w4.append(wp.tile([128, 2, O * V], f16, tag="w", name=f"w4_{c}"))
        nc.sync.dma_start(out=w4[0][:], in_=w2_d[:, 0:2])
        nc.scalar.dma_start(out=xdgA[:], in_=xdg_d[:, 0:16])
        nc.gpsimd.dma_start(out=ones_t[:], in_=ones_d[:])
        nc.gpsimd.dma_start(out=iden_t[:], in_=iden_d[:])
        nc.gpsimd.memset(bias8[:], -8.0)
        nc.gpsimd.dma_start(
            out=bias_t[:], in_=bias_d[:].unsqueeze(0).broadcast_to([BL, V, O])
        )
        nc.scalar.dma_start(out=xdgB[:], in_=xdg_d[:, 16:32])
        qrr = [nc.gpsimd, nc.sync, nc.scalar]
        for c in range(1, 16):
            qrr[c % 3].dma_start(out=w4[c][:], in_=w2_d[:, 2 * c : 2 * c + 2])

        # ---- phase 1: priors + out0 ----
        out0 = ps_o.tile([128, V, O], f32, tag="out")
        out0f = out0[:].rearrange("p v o -> p (v o)")
        # odd-ib partial-sum tree scratch aliases prod's first 8 ib slots
        st = prod[:, 0:8].rearrange("p (k j) v o -> p k j v o", j=2)
        pr2 = priors[:].rearrange("p (g two) v o -> p g two v o", two=2)
        for ib in range(IB):
            xt = xdgA if ib < 16 else xdgB
            pp = ps_a.tile([128, O * V], f32, tag="pp")
            for h in range(2):
                sl = slice(h * 512, (h + 1) * 512)
                nc.tensor.matmul(
                    pp[:, sl], xt[:, ib % 16], w4[ib // 2][:, ib % 2, sl],
                    start=True, stop=True,
                )
            ppv = pp[:].rearrange("p (o v) -> p v o", o=O)
            if CAST_PAT[ib] == "a":
                nc.scalar.copy(out=priors[:, ib], in_=ppv)
            else:
                nc.vector.tensor_copy(out=priors[:, ib], in_=ppv)
            if ib % 2 == 0:
                # even ibs: accumulate out0 on the PE
                pslc = priors[:, ib].rearrange("p v o -> p (v o)")
                for h in range(2):
                    sl = slice(h * 512, (h + 1) * 512)
                    nc.tensor.matmul(
                        out0f[:, sl], ones_t[:], pslc[:, sl],
                        start=(ib == 0), stop=False, skip_group_check=True,
                    )
            if ib % 8 == 7:
                # odd ibs of this window: one paired DVE add, then fold into
                # the running accumulator in st8[:, 0:2] so only one add plus
                # the j-fold remain after the last cast.
                k = ib // 8
                nc.vector.tensor_add(
                    st[:, k], pr2[:, 4 * k : 4 * k + 2, 1],
                    pr2[:, 4 * k + 2 : 4 * k + 4, 1],
                )
                if k > 0:
                    st8 = st[:].rearrange("p k j v o -> p (k j) v o")
                    nc.vector.tensor_add(
                        st8[:, 0:2], st8[:, 0:2], st8[:, 2 * k : 2 * k + 2]
                    )
        st8 = st[:].rearrange("p k j v o -> p (k j) v o")
        nc.vector.tensor_add(st8[:, 0], st8[:, 0], st8[:, 1])
        stf = st8[:, 0].rearrange("p v o -> p (v o)")
        for h in range(2):
            sl = slice(h * 512, (h + 1) * 512)
            nc.tensor.matmul(
                out0f[:, sl], ones_t[:], stf[:, sl],
                start=False, stop=True, skip_group_check=True,
            )

        # ---- routing iterations ----
        # (Tried and reverted: keep-warm dummy PE matmuls across iteration
        # boundaries cost ~10us net -- they delay real matmuls more than the
        # p-state ramp costs. reciprocal_approx_fast gave wrong results on HW.)
        def norm_chain(out_ps, dst_ntile):
            # dst_ntile = out/||out||; sqrt is the only non-exp-table ACT op,
            # its table loads hide in ACT idle gaps.
            src_vo = out_ps[:]
            src_ov = out_ps[:].transpose([0, 2, 1])  # [128, O, V] view
            sq = sm.tile([128, O, V], f32, tag="sq")
            nc.scalar.square(sq[:], src_ov)
            nsq = sm.tile([128, O], f32, tag="nsq")
            nc.vector.tensor_reduce(out=nsq[:], in_=sq[:], axis=X, op=AL.add)
            norm = sm.tile([128, O], f32, tag="norm")
            nc.scalar.sqrt(norm[:], nsq[:])
            rn = sm.tile([128, O], f32, tag="rn")
            nc.vector.reciprocal(rn[:], norm[:])
            nc.vector.tensor_mul(
                dst_ntile[:], src_vo, rn[:].unsqueeze(1).broadcast_to([128, V, O])
            )
            return nsq

        norm_chain(out0, ntile)

        out_prev = out0
        for t in range(3):
            ntb = ntile[:].unsqueeze(1)

            # 5 chunks: a small first chunk halves the cold-PE v-reduce-c0
            # latency so the softmax (and with it mul2) starts earlier.
            CHUNKS = ((0, 4), (4, 12), (12, 20), (20, 28), (28, 32))
            lgt = [
                ps_a.tile([128, 2, QIB * O], f32, tag="pp", name=f"lg{i}_{t}")
                for i in range(3)
            ]

            def lg_of(q, ln):
                return lgt[q // 2][:, q % 2][:, 0 : ln * O]

            def exp_chunk(q, a, b):
                lgq3 = lg_of(q, b - a).rearrange("p (q o) -> p q o", o=O)
                nc.scalar.activation(
                    out=elog[:, a:b], in_=lgq3, func=AF.Exp, bias=bias8[:]
                )

            def zs_chunk(q, a, b):
                with nc.allow_low_precision(
                    reason="DVE reduces in fp32 internally; fp16 elog input"
                ):
                    nc.vector.tensor_reduce(
                        out=zs[:, a:b], in_=elog[:, a:b], axis=X, op=AL.add
                    )
                nc.vector.reciprocal(rz[:, a:b], zs[:, a:b])

            def probs_chunk(q, a, b):
                # probs = elog * (1/zs): per-ib on ACT -- the 1/zs factor is a
                # per-partition scalar once sliced per ib, freeing the DVE.
                for ib in range(a, b):
                    nc.scalar.activation(
                        out=probs[:, ib], in_=elog[:, ib], func=AF.Copy,
                        bias=0.0, scale=rz[:, ib : ib + 1],
                    )

            def vred_chunk(q, a, b):
                lgq = lg_of(q, b - a)
                pq = prod[:, a:b]
                for v in range(V):
                    nc.tensor.matmul(
                        lgq, iden_t[:], pq[:, :, v],
                        start=(v == 0), stop=(v == V - 1), skip_group_check=True,
                    )

            for q, (a, bnd) in enumerate(CHUNKS):
                nc.vector.tensor_mul(
                    prod[:, a:bnd], priors[:, a:bnd],
                    ntb.broadcast_to([128, bnd - a, V, O]),
                )
                vred_chunk(q, a, bnd)
                exp_chunk(q, a, bnd)
                zs_chunk(q, a, bnd)
                probs_chunk(q, a, bnd)

            # mul2 + i-reduce trailing each DVE slice
            out_new = ps_o.tile([128, V, O], f32, tag="out")
            onf = out_new[:].rearrange("p v o -> p (v o)")
            for a, bnd in DVE_M2_SLICES:
                nc.vector.tensor_mul(
                    prod[:, a:bnd], priors[:, a:bnd],
                    probs[:, a:bnd].unsqueeze(2).broadcast_to([128, bnd - a, V, O]),
                )
                for ib in range(a, bnd):
                    pslc = prod[:, ib].rearrange("p v o -> p (v o)")
                    for h in range(2):
                        sl = slice(h * 512, (h + 1) * 512)
                        nc.tensor.matmul(
                            onf[:, sl], ones_t[:], pslc[:, sl],
                            start=(ib == 0), stop=(ib == IB - 1),
                            skip_group_check=True,
                        )

            if t < 2:
                norm_chain(out_new, ntile)
            out_prev = out_new

        # ---- squash + bias on partitions 0..7 (b rows) ----
        sq2 = sm.tile([128, O, V], f32, tag="sq")
        src_ov = out_prev[:].transpose([0, 2, 1])
        nc.scalar.square(sq2[:], src_ov)
        nsq2 = sm.tile([128, O], f32, tag="nsq")
        nc.vector.tensor_reduce(out=nsq2[:], in_=sq2[:], axis=X, op=AL.add)
        norm2 = sm.tile([128, O], f32, tag="norm")
        nc.scalar.sqrt(norm2[:], nsq2[:])
        den = sm.tile([128, O], f32, tag="den")
        nc.vector.tensor_scalar_add(den[:], nsq2[:], 1.0)
        rden = sm.tile([128, O], f32, tag="rden")
        nc.vector.reciprocal(rden[:], den[:])
        scl = sm.tile([128, O], f32, tag="scl")
        nc.vector.tensor_mul(scl[:], norm2[:], rden[:])

        outf = sm.tile([BL, V, O], f32, tag="outf")
        nc.vector.tensor_mul(
            outf[:],
            out_prev[0:BL],
            scl[0:BL].unsqueeze(1).broadcast_to([BL, V, O]),
        )
        nc.vector.tensor_add(outf[:], outf[:], bias_t[:])
        nc.sync.dma_start(out=out_d[:], in_=outf[:])


_NC_CACHE = []


def _get_nc():
    if not _NC_CACHE:
        _NC_CACHE.append(_build_kernel())
    return _NC_CACHE[0]


def kernel(x, weight, bias):
    global LAST_RESULT
    x = np.asarray(x, dtype=np.float32)
    weight = np.asarray(weight, dtype=np.float32)
    bias = np.asarray(bias, dtype=np.float32)

    # W2[(i_sub, l), ib, (o, v)] = W[o, ib*16+i_sub, v, l]
    w2 = np.ascontiguousarray(
        weight.transpose(1, 3, 0, 2)
        .reshape(IB, 16, L, O * V)
        .transpose(1, 2, 0, 3)
        .reshape(128, IB, O * V)
    ).astype(np.float16)
    biasT = np.ascontiguousarray(bias.T)  # [V, O]

    idx = np.arange(128)
    onesd = (idx[:, None] % BL == idx[None, :] % BL).astype(np.float16)
    idend = np.eye(128, dtype=np.float16)

    in_maps = []
    for c in range(NCORES):
        xc = x[c * BL : (c + 1) * BL]  # [BL, I, L]
        xt = np.ascontiguousarray(xc.transpose(1, 2, 0))  # [I, L, BL]
        xt4 = xt.reshape(IB, 16, L, BL)
        xdg = np.zeros((IB, 128, 128), dtype=np.float16)
        for s in range(16):
            xdg[:, s * L : (s + 1) * L, s * BL : (s + 1) * BL] = xt4[:, s].astype(
                np.float16
            )
        xdg = np.ascontiguousarray(xdg.transpose(1, 0, 2))  # [128, IB, 128]
        in_maps.append(
            {"w2": w2, "xdg": xdg, "onesd": onesd, "idend": idend, "biasT": biasT}
        )

    nc = _get_nc()
    try:
        res = run_bass_kernel_spmd(nc, in_maps, core_ids=list(range(NCORES)))
    except ModuleNotFoundError:
        os.environ["BASS_NEVER_TRACE"] = "1"
        res = run_bass_kernel_spmd(nc, in_maps, core_ids=list(range(NCORES)))
    LAST_RESULT = res

    outs = []
    for r in res.results:
        o = r["out"]  # [BL, V, O]
        outs.append(np.ascontiguousarray(o.transpose(0, 2, 1)))  # [BL, O, V]
    return np.concatenate(outs, axis=0).astype(np.float32)


if __name__ == "__main__":
    rng = np.random.default_rng(0)
    x = rng.standard_normal((B, I, L), dtype=np.float32)
    w = rng.standard_normal((O, I, V, L), dtype=np.float32) * 0.1
    b = rng.standard_normal((O, V), dtype=np.float32) * 0.1
    out = kernel(x, w, b)
    print("out shape", out.shape, out.dtype)
